# revision 2
# baseline (speedup 1.0000x reference)
"""GraphConv 2-layer GNN on 8 Trainium2 NeuronCores — fused-slab version.

Layer 1 fuses the segment-sum into the W1_rel matmul: the host lays out the
edge payload as columns of x.T (features on partitions, one column per edge),
grouped into degree-sorted prefix "slabs" (slab j = the j-th neighbor
contribution, which by descending-degree order covers a contiguous prefix of
node ranks).  An accumulating PE matmul chain with stationary W1_rel.T
computes W1_rel @ segment_sum(x[src]) directly in PSUM per 512-node stripe.
A subset of stripes ("DVE stripes", chosen by a cost balancer) instead carry
a windowed [f, node, d] payload that the vector engine reduces to agg.T,
feeding a single matmul — splitting the reduction across both engines.  The
W1_root @ x.T term reads own-node x.T columns from an SBUF-resident tile
(loaded once, outside the steady-state loop), so it costs no HBM traffic.
relu+bias yields h.T, and a packed [128,104] weight computes y2.T =
W2_rel@h.T (the 40-wide layer-2 edge features) and r2.T = W2_root@h.T in one
matmul; a single scalar-engine copy moves both PSUM halves into a merged
[104, OWNP] fp8 tile.

Layer 2 aggregates y2[src] with a windowed layout at 128-node granularity:
window w's depth D_w is its max degree, and consecutive depth slots are
PAIRED into fp8 DoubleRow identity matmuls (two depth blocks per PE pass,
2x column rate).  Depth blocks over a chunk of 12 windows shrink as prefixes
of the PSUM stripe, exactly like layer 1's slabs.  The r2 term is preloaded
into SBUF, and log-softmax runs along the free axis with exp batched
per-chunk and a single ln at the end.

Payloads are fp8e4m3 (PSUM accumulation stays exact f32; only quantization
error enters).  Host work is layout/permutation prep only.
"""
import sys
sys.path.insert(0, "/opt/trn_rl_repo")
import numpy as np
import ml_dtypes

import concourse.bacc as bacc
import concourse.mybir as mybir
import concourse.tile as tile
from concourse.bass_utils import run_bass_kernel_spmd

BF16 = ml_dtypes.bfloat16
FP8 = ml_dtypes.float8_e4m3
N, E, F, H, C = 100000, 1600000, 128, 128, 40
NCORES = 8
OWN = N // NCORES          # 12500 dst nodes per core
P = 128
STRIPE = 512               # nodes per PSUM stripe (launch 1)
NSTR = (OWN + STRIPE - 1) // STRIPE    # 25
OWNP = NSTR * STRIPE       # 12800
NW = OWNP // P             # 100 layer-2 windows of 128 nodes
CHW = 12                   # windows per layer-2 chunk (PSUM: 12*40 f32 < 2KB)
ALIGN = 16                 # pay1 piece width alignment (cols)

# engine cost constants for the stripe balancer (ns per payload column)
PE_NS_PER_COL = 0.4714     # measured, launch1-shaped bf16 chain
DVE_NS_PER_COL = 1.04      # measured
PE_FIXED_COLS = 2 * 512 * NSTR   # root + tail matmuls per stripe

BF = mybir.dt.bfloat16
F8 = mybir.dt.float8e4
F32 = mybir.dt.float32
DRMODE = mybir.MatmulPerfMode.DoubleRow


class Sched:
    pass


def _prep_graph(edge_index):
    """Shared (cross-core) schedule + per-core edge->column maps."""
    src = np.asarray(edge_index[0], dtype=np.int64)
    dst = np.asarray(edge_index[1], dtype=np.int64)
    deg = np.bincount(dst, minlength=N)

    orders = []           # per core: global node ids in degree-desc order
    degs = np.zeros((NCORES, OWNP), np.int64)
    for c in range(NCORES):
        ids = np.arange(c * OWN, (c + 1) * OWN)
        o = ids[np.argsort(-deg[ids], kind="stable")]
        orders.append(o)
        degs[c, :OWN] = deg[o]

    Jmax = int(degs.max())
    n = (degs[:, None, :] > np.arange(Jmax)[None, :, None]).sum(2)  # [8, Jmax]

    # per-stripe PE piece widths and DVE depths
    Ws = []
    Ds = np.zeros(NSTR, np.int64)
    for s in range(NSTR):
        a = s * STRIPE
        w = np.clip(np.minimum(n, (s + 1) * STRIPE) - a, 0, STRIPE).max(0)
        w = w[w > 0]
        w = np.minimum((w + ALIGN - 1) // ALIGN * ALIGN, STRIPE)
        lst = w.tolist()
        if not lst:
            lst = [STRIPE]
        lst[0] = STRIPE
        Ws.append(lst)
        Ds[s] = max(1, int(degs[:, a].max()))

    # balance stripes between PE (slab chain) and DVE (windowed reduce)
    pe_cols = np.array([sum(w) for w in Ws], np.float64)
    mode = ["pe"] * NSTR
    pe_t = PE_NS_PER_COL * (pe_cols.sum() + PE_FIXED_COLS)
    dve_t = 0.0
    cand = sorted(range(NSTR), key=lambda s: -(pe_cols[s]) / (512 * Ds[s]))
    for s in cand:
        gain = PE_NS_PER_COL * (pe_cols[s] - 512)
        cost = DVE_NS_PER_COL * 512 * (Ds[s] + 1)   # windows + aggB copy
        if max(pe_t - gain, dve_t + cost) < max(pe_t, dve_t):
            mode[s] = "dve"
            pe_t -= gain
            dve_t += cost

    # column layout: stripe-major; per stripe either PE pieces or a DVE window
    stripe_cols = np.zeros(NSTR, np.int64)
    for s in range(NSTR):
        body = 512 * int(Ds[s]) if mode[s] == "dve" else int(sum(Ws[s]))
        stripe_cols[s] = body
    stripe_off = np.zeros(NSTR + 1, np.int64)
    stripe_off[1:] = np.cumsum(stripe_cols)
    C1 = int(stripe_off[-1])

    CB = np.full((NSTR, Jmax + 1), -1, np.int64)
    for s in range(NSTR):
        if mode[s] == "pe":
            loc = int(stripe_off[s])
            for j, w in enumerate(Ws[s]):
                CB[s, j] = loc
                loc += w
    is_dve = np.array([m == "dve" for m in mode])

    # ---- launch-2 schedule: per-window depths, chunked, depth-paired ----
    wmax = np.maximum(degs[:, ::P].max(0), 1).astype(np.int64)   # [NW]
    nchunks = (NW + CHW - 1) // CHW
    ck_nw = [min(CHW, NW - k * CHW) for k in range(nchunks)]
    ck_pairs = []          # per chunk: list of (n0, n1) window counts per pair
    ck_base = []           # per chunk: per-pair base offset (C-units)
    tot40 = 0
    for k in range(nchunks):
        wlo = k * CHW
        Dw = wmax[wlo:wlo + ck_nw[k]]
        Dk = int(Dw[0])
        pairs, bases = [], []
        for dp in range((Dk + 1) // 2):
            n0 = int((Dw > 2 * dp).sum())
            n1 = int((Dw > 2 * dp + 1).sum())
            pairs.append((n0, n1))
            bases.append(tot40)
            tot40 += 2 * n0
        ck_pairs.append(pairs)
        ck_base.append(bases)
    C2_40 = tot40

    # per-edge col40: dp = j>>1, phase = j&1
    pairbase = np.full((nchunks, (int(wmax[0]) + 1) // 2), -1, np.int64)
    pairn0 = np.zeros_like(pairbase)
    for k in range(nchunks):
        for dp, (n0, _) in enumerate(ck_pairs[k]):
            pairbase[k, dp] = ck_base[k][dp]
            pairn0[k, dp] = n0

    core = dst // OWN
    cmaps = []
    for c in range(NCORES):
        rank_of = np.empty(OWN, np.int64)
        rank_of[orders[c] - c * OWN] = np.arange(OWN)
        m = core == c
        s_c, d_c = src[m], dst[m]
        r = rank_of[d_c - c * OWN]
        perm = np.argsort(r, kind="stable")
        r_s = r[perm]
        s_s = s_c[perm]
        first = np.searchsorted(r_s, r_s)
        j = np.arange(len(r_s)) - first
        se = r_s >> 9
        rl = r_s & (STRIPE - 1)
        col_pe = CB[se, j] + rl
        col_dve = stripe_off[se] + rl * Ds[se] + j
        col1 = np.where(is_dve[se], col_dve, col_pe)
        w = r_s >> 7
        k = w // CHW
        wi = w - k * CHW
        col40 = pairbase[k, j >> 1] + (j & 1) * pairn0[k, j >> 1] + wi
        p2 = r_s & (P - 1)
        cmaps.append((s_s, col1, p2, col40))

    sc = Sched()
    sc.orders, sc.degs = orders, degs
    sc.Ws, sc.stripe_off, sc.C1 = Ws, stripe_off, C1
    sc.mode, sc.Ds = mode, Ds
    sc.nchunks, sc.ck_nw, sc.ck_pairs, sc.ck_base = nchunks, ck_nw, ck_pairs, ck_base
    sc.C2_40 = C2_40
    sc.cmaps = cmaps
    return sc


def _inputs1(sc, x, W1_rel, b1, W1_root, W2_rel, W2_root):
    x8 = np.asarray(x, np.float32).astype(FP8)
    w1relT = np.ascontiguousarray(np.asarray(W1_rel, np.float32).T).astype(BF16)
    w1rootT = np.ascontiguousarray(np.asarray(W1_root, np.float32).T).astype(BF16)
    w2pT = np.zeros((H, 64 + C), np.float32)
    w2pT[:, :C] = np.asarray(W2_rel, np.float32).T
    w2pT[:, 64:] = np.asarray(W2_root, np.float32).T
    w2pT = w2pT.astype(BF16)
    b1v = np.zeros((P, 1), np.float32)
    b1v[:H, 0] = np.asarray(b1, np.float32)
    in_maps = []
    for c in range(NCORES):
        s_s, col1, _, _ = sc.cmaps[c]
        pay_cm = np.zeros((sc.C1, F), FP8)
        pay_cm[col1] = x8[s_s]
        pay1 = np.ascontiguousarray(pay_cm.T)
        xo = np.zeros((OWNP, F), FP8)
        xo[:OWN] = x8[sc.orders[c]]
        xoT = np.ascontiguousarray(xo.T)
        in_maps.append({"pay1": pay1, "xo": xoT, "w1relT": w1relT,
                        "w1rootT": w1rootT, "w2pT": w2pT, "b1v": b1v})
    return in_maps


def _inputs2(sc, y2s, r2s, b2):
    """Launch-2 inputs from launch-1 outputs y2o [C,OWNP] / r2o [C,OWNP]."""
    b2f = np.asarray(b2, np.float32)
    y2g = np.zeros((N, C), FP8)
    for c in range(NCORES):
        y2g[sc.orders[c]] = y2s[c][:, :OWN].T
    ide = np.eye(P, dtype=FP8)
    identD = np.ascontiguousarray(np.concatenate([ide, ide], axis=1))
    in_maps = []
    for c in range(NCORES):
        s_s, _, p2, col40 = sc.cmaps[c]
        pay = np.zeros((P, sc.C2_40, C), FP8)
        pay[p2, col40] = y2g[s_s]
        r2 = r2s[c].astype(np.float32).T + b2f               # [OWNP, 40]
        r2b = np.ascontiguousarray(
            r2.reshape(NW, P, C).transpose(1, 0, 2)).astype(BF16)
        in_maps.append({"pay2": pay.reshape(P, -1),
                        "r2b": r2b.reshape(P, -1), "identD": identD})
    return in_maps


def _build1(sc, R=1):
    nc = bacc.Bacc()
    pay1 = nc.declare_dram_parameter("pay1", [P, sc.C1], F8, isOutput=False)
    xo = nc.declare_dram_parameter("xo", [P, OWNP], F8, isOutput=False)
    w1relT = nc.declare_dram_parameter("w1relT", [F, H], BF, isOutput=False)
    w1rootT = nc.declare_dram_parameter("w1rootT", [F, H], BF, isOutput=False)
    w2pT = nc.declare_dram_parameter("w2pT", [H, 64 + C], BF, isOutput=False)
    b1v = nc.declare_dram_parameter("b1v", [P, 1], F32, isOutput=False)
    y2o = nc.declare_dram_parameter("y2o", [C, OWNP], F8, isOutput=True)
    r2o = nc.declare_dram_parameter("r2o", [C, OWNP], F8, isOutput=True)

    with tile.TileContext(nc) as tc:
        with (
            tc.tile_pool(name="const", bufs=1) as cpool,
            tc.tile_pool(name="stream", bufs=3) as spool,
            tc.tile_pool(name="h", bufs=3) as hpool,
            tc.tile_pool(name="agg", bufs=2) as apool,
            tc.tile_pool(name="dagg", bufs=1) as dpool,
            tc.tile_pool(name="yrst", bufs=1) as ypool,
            tc.tile_pool(name="ph", bufs=2, space="PSUM") as php,
            tc.tile_pool(name="py", bufs=2, space="PSUM") as pyp,
        ):
            w1rel_t = cpool.tile([F, H], BF)
            nc.sync.dma_start(out=w1rel_t[:], in_=w1relT[:])
            w1root_t = cpool.tile([F, H], BF)
            nc.sync.dma_start(out=w1root_t[:], in_=w1rootT[:])
            w2p_t = cpool.tile([H, 64 + C], BF)
            nc.sync.dma_start(out=w2p_t[:], in_=w2pT[:])
            b1_t = cpool.tile([P, 1], F32)
            nc.sync.dma_start(out=b1_t[:], in_=b1v[:])
            xo_t = cpool.tile([P, OWNP], F8)
            nc.sync.dma_start(out=xo_t[:], in_=xo[:])

            def tail(s, ph, ysb):
                a = s * STRIPE
                ht = hpool.tile([P, STRIPE], BF, tag="ht")
                nc.scalar.activation(
                    out=ht[:], in_=ph[:],
                    func=mybir.ActivationFunctionType.Relu,
                    bias=b1_t[:, :1], scale=1.0)
                py = pyp.tile([64 + C, STRIPE], F32, tag="py")
                nc.tensor.matmul(out=py[:], lhsT=w2p_t[:], rhs=ht[:],
                                 start=True, stop=True)
                nc.scalar.activation(
                    out=ysb[:, a:a + STRIPE], in_=py[:],
                    func=mybir.ActivationFunctionType.Copy, scale=1.0)

            def body(_iv=None):
                ysb = ypool.tile([64 + C, OWNP], F8, tag="ysb")
                pe_list = [s for s in list(range(NSTR))[::-1]
                           if sc.mode[s] == "pe"]
                dve_list = [s for s in list(range(NSTR))[::-1]
                            if sc.mode[s] == "dve"]
                # phase 1: PE stripes run their full pipeline; DVE stripes'
                # payloads stream+reduce early, spread between PE stripes so
                # neither the DMA queue nor DVE is bursty.  DVE-dependent
                # matmuls are deferred to phase 2 so PE never stalls on DVE.
                aggBs = {}
                sched = []
                step = max(1, len(pe_list) // max(1, len(dve_list)))
                di = 0
                for i, s in enumerate(pe_list):
                    sched.append(("pe", s))
                    if i % step == 0 and di < len(dve_list):
                        sched.append(("dve", dve_list[di]))
                        di += 1
                while di < len(dve_list):
                    sched.append(("dve", dve_list[di]))
                    di += 1
                for si, (md, s) in enumerate(sched):
                    c0, c1 = int(sc.stripe_off[s]), int(sc.stripe_off[s + 1])
                    a = s * STRIPE
                    st = spool.tile([P, c1 - c0], F8, tag="pay")
                    nc.sync.dma_start(out=st[:], in_=pay1[:, c0:c1])
                    if md == "dve":
                        D = int(sc.Ds[s])
                        aggF = apool.tile([P, STRIPE, 1], F32, tag="aggF")
                        CH = 128
                        for k in range(0, STRIPE, CH):
                            nc.vector.reduce_sum(
                                out=aggF[:, k:k + CH, :],
                                in_=st[:, k * D:(k + CH) * D].rearrange(
                                    "p (n d) -> p n d", d=D),
                                axis=mybir.AxisListType.X)
                        aggB = dpool.tile([P, STRIPE], BF, tag=f"aggB{s}")
                        nc.vector.tensor_copy(
                            out=aggB[:],
                            in_=aggF[:].rearrange("p n o -> p (n o)"))
                        aggBs[s] = aggB
                        continue
                    ph = php.tile([P, STRIPE], F32, tag="ph")
                    loc = 0
                    for j, w in enumerate(sc.Ws[s]):
                        nc.tensor.matmul(
                            out=ph[:, :w], lhsT=w1rel_t[:],
                            rhs=st[:, loc:loc + w],
                            start=(j == 0), stop=False)
                        loc += w
                    nc.tensor.matmul(
                        out=ph[:], lhsT=w1root_t[:],
                        rhs=xo_t[:, a:a + STRIPE],
                        start=False, stop=True)
                    tail(s, ph, ysb)
                # phase 2: DVE stripes' matmul pipelines (aggB ready long ago)
                for s in dve_list:
                    a = s * STRIPE
                    ph = php.tile([P, STRIPE], F32, tag="ph")
                    nc.tensor.matmul(out=ph[:], lhsT=w1rel_t[:],
                                     rhs=aggBs[s][:], start=True, stop=False)
                    nc.tensor.matmul(out=ph[:], lhsT=w1root_t[:],
                                     rhs=xo_t[:, a:a + STRIPE],
                                     start=False, stop=True)
                    tail(s, ph, ysb)
                nc.sync.dma_start(out=y2o[:], in_=ysb[:C, :])
                nc.sync.dma_start(out=r2o[:], in_=ysb[64:64 + C, :])

            if R > 1:
                with tc.For_i(0, R, 1):
                    body()
            else:
                body()
    nc.finalize()
    return nc


def _build2(sc, R=1):
    nc = bacc.Bacc()
    pay2 = nc.declare_dram_parameter("pay2", [P, sc.C2_40 * C], F8, isOutput=False)
    r2b = nc.declare_dram_parameter("r2b", [P, NW * C], BF, isOutput=False)
    identD = nc.declare_dram_parameter("identD", [P, 2 * P], F8, isOutput=False)
    out = nc.declare_dram_parameter("out", [P, NW * C], BF, isOutput=True)
    nch = sc.nchunks

    with tile.TileContext(nc) as tc:
        with (
            tc.tile_pool(name="const", bufs=1) as cpool,
            tc.tile_pool(name="stream", bufs=3) as spool,
            tc.tile_pool(name="work", bufs=3) as wpool,
            tc.tile_pool(name="stash", bufs=1) as tpool,
            tc.tile_pool(name="ost", bufs=1) as opool,
            tc.tile_pool(name="ps", bufs=3, space="PSUM") as ppool,
        ):
            ident2 = cpool.tile([P, 2, P], F8)
            nc.sync.dma_start(
                out=ident2[:].rearrange("p t m -> p (t m)"), in_=identD[:])
            r2_t = cpool.tile([P, NW, C], BF)
            nc.sync.dma_start(
                out=r2_t[:].rearrange("p w c -> p (w c)"), in_=r2b[:])

            def body(_iv=None):
                smst = tpool.tile([P, NW, C], F32, tag="smst")
                mxst = tpool.tile([P, NW, 1], F32, tag="mxst")
                smest = tpool.tile([P, NW, 1], F32, tag="smest")
                for k in range(nch):
                    nwk = sc.ck_nw[k]
                    wlo = k * CHW
                    pairs = sc.ck_pairs[k]
                    o0 = sc.ck_base[k][0] * C
                    oend = (sc.ck_base[k][-1] + 2 * pairs[-1][0]) * C
                    st = spool.tile([P, oend - o0], F8, tag="pay")
                    nc.sync.dma_start(out=st[:], in_=pay2[:, o0:oend])
                    ps = ppool.tile([P, nwk * C], F32, tag="agg")
                    for dp, (n0, _) in enumerate(pairs):
                        b = sc.ck_base[k][dp] * C - o0
                        nc.tensor.matmul(
                            out=ps[:, :n0 * C], lhsT=ident2[:],
                            rhs=st[:, b:b + 2 * n0 * C].rearrange(
                                "p (t n) -> p t n", t=2),
                            start=(dp == 0), stop=(dp == len(pairs) - 1),
                            perf_mode=DRMODE)
                    ksl = smst[:, wlo:wlo + nwk]
                    nc.vector.tensor_add(
                        out=ksl.rearrange("p w c -> p (w c)"),
                        in0=ps[:],
                        in1=r2_t[:, wlo:wlo + nwk].rearrange("p w c -> p (w c)"))
                    nc.vector.reduce_max(out=mxst[:, wlo:wlo + nwk], in_=ksl,
                                         axis=mybir.AxisListType.X)
                    ex = wpool.tile([P, nwk, C], BF, tag="ex")
                    nc.vector.tensor_tensor(
                        out=ex[:], in0=ksl,
                        in1=mxst[:, wlo:wlo + nwk].to_broadcast([P, nwk, C]),
                        op=mybir.AluOpType.subtract)
                    nc.scalar.activation(
                        out=ex[:], in_=ex[:],
                        func=mybir.ActivationFunctionType.Exp)
                    nc.vector.reduce_sum(out=smest[:, wlo:wlo + nwk], in_=ex[:],
                                         axis=mybir.AxisListType.X)
                ls = wpool.tile([P, NW], F32, tag="ls")
                nc.scalar.activation(
                    out=ls[:], in_=smest[:].rearrange("p w o -> p (w o)"),
                    func=mybir.ActivationFunctionType.Ln)
                tot = wpool.tile([P, NW, 1], F32, tag="tot")
                nc.vector.tensor_add(
                    out=tot[:].rearrange("p w o -> p (w o)"),
                    in0=mxst[:].rearrange("p w o -> p (w o)"), in1=ls[:])
                ost = opool.tile([P, NW, C], BF, tag="ost")
                nc.vector.tensor_tensor(
                    out=ost[:], in0=smst[:],
                    in1=tot[:].to_broadcast([P, NW, C]),
                    op=mybir.AluOpType.subtract)
                nc.sync.dma_start(
                    out=out[:], in_=ost[:].rearrange("p w c -> p (w c)"))

            if R > 1:
                with tc.For_i(0, R, 1):
                    body()
            else:
                body()
    nc.finalize()
    return nc


def kernel(x, edge_index, W1_rel, b1, W1_root, W2_rel, b2, W2_root):
    sc = _prep_graph(edge_index)
    nc1 = _build1(sc)
    nc2 = _build2(sc)

    in1 = _inputs1(sc, x, W1_rel, b1, W1_root, W2_rel, W2_root)
    res1 = run_bass_kernel_spmd(nc1, in1, list(range(NCORES)))
    y2s = [res1.results[c]["y2o"] for c in range(NCORES)]
    r2s = [res1.results[c]["r2o"] for c in range(NCORES)]

    in2 = _inputs2(sc, y2s, r2s, b2)
    res2 = run_bass_kernel_spmd(nc2, in2, list(range(NCORES)))

    out = np.zeros((N, C), np.float32)
    for c in range(NCORES):
        o = res2.results[c]["out"].astype(np.float32).reshape(P, NW, C)
        o = o.transpose(1, 0, 2).reshape(OWNP, C)[:OWN]
        out[sc.orders[c]] = o
    return out
